# revision 19
# baseline (speedup 1.0000x reference)
"""ArcFace-style margin softmax CE loss on 8 Trainium2 cores.

Math: the reference is mean softmax-CE over logits = 64*clip(cos_theta)
with the label column replaced by 64*(ct*cos(m) - sqrt(1-ct^2)*sin(m)).
Since cos_theta lives in [0,1), every exponent 64*x - 64 is <= 0, so a
fixed offset of 64 replaces the per-row max of the log-sum-exp.  The
device then only needs per-row sums of exp(64*x - 64) over the
2048x50000 matrix — a pure streaming, memory-bound reduction.  The
label-column swap (one element per row) and the final mean are O(B)
and are done on the host in f64.

Sharding: data-parallel over rows, 256 rows per core (contiguous
slices of the input, zero host-side copies, no cross-core combine).

Kernel (per core, raw Bass — one semaphore wait per instruction, which
is all this walrus build's codegen accepts):
  sync  : stream chunks [128 x w] HBM->SBUF, NBUF-deep rotation, then
          a split result store (all-but-last column early, [128 x 1]
          after the final ACT)
  scalar: ACTIVATE Exp(64x-64) in place with accum_out -> per-chunk
          row-sums
The per-chunk partial sums [128 x TOTAL] go straight to DRAM and the
final per-row adds run on the host in f64 — no DVE stage on the
critical path.  The ACT is in-place (out == in buffer), freeing SBUF
so chunks can be wide (fewer instructions, same streamed bytes).
Waits ride attached to the consuming instructions, not as separate
sequencer ops.

Tail shaping: the DMA bus (modeled 360 GB/s, globally exclusive) is
saturated end-to-end; total time = preamble + total-bytes/bw + tail.
The tail is the last chunk's DMA-completion semaphore (900 ns), its
ACT, the [128 x 1] store issue (HWDGE 625 + DGE delay 650), and that
store's own mandatory completion semaphore (900 ns — walrus rejects
DMAs without an update).  The end of the stream tapers geometrically
so the ACT chain that remains after the last DMA lands is as short as
possible (per-ACT fixed cost ~370 ns makes very narrow chunks
counterproductive; the taper bottoms out near ~650 columns).
Explored and rejected: prepared SWDGE stores fired by TriggerDma
(kv_writeback / scatter-add skip the HWDGE issue path in the model,
but this walrus/axon build miscompiles or fails them at runtime),
multi-engine DMA issue (the modeled bus is a single exclusive
device), and DMA-transpose loads (2-byte dtypes only, lower modeled
bandwidth).
"""

import contextlib

import numpy as np

import concourse.bass as bass
import concourse.mybir as mybir
from concourse.bass_utils import run_bass_kernel_spmd

B, C = 2048, 50000
N_CORES = 8
RPC = B // N_CORES          # 256 rows per core
P = 128                     # SBUF partitions
ROW_TILES = RPC // P        # 2
SCALE = 64.0
EPS = 1e-7
NBUF = 8                    # input buffer rotation depth

# chunk widths per row tile; rt1 tapers geometrically so the ACT chain
# left after the final DMA is minimal (tuned against the cost model)
W0 = [6250] * 8                                    # row tile 0: uniform
W1 = [5632] * 5 + [5630, 4478, 2885, 1952, 1405,
                   1085, 897, 787, 723, 685, 663, 650]
assert sum(W0) == C and sum(W1) == C

_NC = None                  # cached Bass module (compiled once per process)
LAST_RESULTS = None         # BassKernelResults of the most recent run


def _chunk_table(w0=None, w1=None):
    """[(row_tile, col_start, width)] in stream order."""
    out = []
    for rt, ws in ((0, w0 or W0), (1, w1 or W1)):
        col = 0
        for w in ws:
            out.append((rt, col, w))
            col += w
    return out


def _build(w0=None, w1=None):
    w0 = w0 or W0
    w1 = w1 or W1
    chunks = _chunk_table(w0, w1)
    total = len(chunks)
    wmax = max(w[2] for w in chunks)

    # monotonic_sem_count=0: we don't use MonotonicSemaphores, and skipping
    # their gpsimd preamble ops shortens the init barrier slightly.
    nc = bass.Bass(monotonic_sem_count=0)
    # activation() lowers a float bias to a const AP; -64.0 isn't in the
    # built-in const database, so register it the same way Bass init does
    # (but guard the first ACT with a semaphore instead of a full barrier
    # so the DMA stream starts immediately).
    cneg = nc.alloc_sbuf_tensor("const-float32-neg64", [P, 1], mybir.dt.float32)
    nc.const_aps.aps[(mybir.dt.float32, -SCALE)] = cneg.ap()

    x = nc.dram_tensor("x", [RPC, C], mybir.dt.float32, kind="ExternalInput")
    # 4D [batch=1, d_head_inner=P, d_head_outer=1, n_ctx=total] so the
    # result store can be a prepared kv_writeback (SWDGE) fired by a cheap
    # TriggerDma right after the last ACT — that skips the ~1.3us HWDGE
    # issue path (SEQ+descriptor-gen+DGE delay) a plain dma_start pays.
    s = nc.dram_tensor("s", [P, total], mybir.dt.float32,
                       kind="ExternalOutput")

    bufs = [
        nc.alloc_sbuf_tensor(f"buf{b}", [P, wmax], mybir.dt.float32)
        for b in range(NBUF)
    ]
    partials = nc.alloc_sbuf_tensor("partials", [P, total], mybir.dt.float32)

    def chunk_src(i):
        rt, col, w = chunks[i]
        return x[rt * P:(rt + 1) * P, col:col + w]

    with (
        nc.semaphore("sem_const") as sem_const,
        nc.semaphore("sem_act") as sem_act,
        nc.semaphore("sem_out") as sem_out,
        contextlib.ExitStack() as st,
    ):
        sem_buf = [st.enter_context(nc.semaphore(f"sem_buf{b_}"))
                   for b_ in range(NBUF)]

        with nc.Block() as block:

            @block.gpsimd
            def _(gpsimd):
                gpsimd.memset(cneg.ap(), -SCALE).then_inc(sem_const, 1)

            @block.sync
            def _(sync):
                for i in range(total):
                    b = i % NBUF
                    if i >= NBUF:
                        # slot reuse: ACT #(i-NBUF) has consumed bufs[b]
                        sync.wait_ge(sem_act, i - NBUF + 1)
                    sync.dma_start(
                        out=bufs[b].ap()[:, :chunks[i][2]], in_=chunk_src(i)
                    ).then_inc(sem_buf[b], 16)
                # split result store: everything but the last column goes
                # out while the final ACT still runs (its transfer + sem
                # land in the bus-idle tail); only a [128 x 1] store — and
                # the mandatory 900ns DMA-completion semaphore — remains
                # on the critical path after the last ACT.  Waits ride on
                # the DMA instructions (no separate sequencer waits).
                sync.dma_start(out=s[:, :total - 1],
                               in_=partials.ap()[:, :total - 1]
                               )._wait_ge(sem_act, total - 1
                                          ).then_inc(sem_out, 16)
                with nc.allow_non_contiguous_dma(
                        reason="[128x1] column store, 128 tiny descriptors"):
                    sync.dma_start(out=s[:, total - 1:],
                                   in_=partials.ap()[:, total - 1:]
                                   )._wait_ge(sem_act, total
                                              ).then_inc(sem_out, 16)

            @block.scalar
            def _(scalar):
                scalar.wait_ge(sem_const, 1)
                for i in range(total):
                    b = i % NBUF
                    w = chunks[i][2]
                    # wait rides on the ACT: the (i//NBUF + 1)-th DMA into
                    # this slot is done; slot DMAs are serialized by the
                    # ACT chain itself, so this per-slot count is race-free.
                    scalar.activation(
                        bufs[b].ap()[:, :w],
                        bufs[b].ap()[:, :w],
                        mybir.ActivationFunctionType.Exp,
                        bias=-SCALE,
                        scale=SCALE,
                        accum_out=partials.ap()[:, i:i + 1],
                    )._wait_ge(sem_buf[b], 16 * (i // NBUF + 1)
                               ).then_inc(sem_act, 1)

    return nc


def kernel(cos_theta, labels, margins):
    global _NC, LAST_RESULTS
    ct = np.ascontiguousarray(np.asarray(cos_theta, dtype=np.float32))
    lab = np.asarray(labels).astype(np.int64)
    mg = np.asarray(margins, dtype=np.float64)
    assert ct.shape == (B, C)

    if _NC is None:
        _NC = _build()

    n0 = len(W0)
    total = n0 + len(W1)
    in_maps = [{"x": ct[i * RPC:(i + 1) * RPC]} for i in range(N_CORES)]
    LAST_RESULTS = run_bass_kernel_spmd(_NC, in_maps, list(range(N_CORES)))
    # s[p, i] is chunk i's partial row-sum for global row
    # core*RPC + rt(i)*P + p; finish the reduction here in f64
    S_parts = []
    for i in range(N_CORES):
        ps = LAST_RESULTS.results[i]["s"].astype(np.float64)  # [P, total]
        S_parts.append(ps[:, :n0].sum(axis=1))        # rows rt0
        S_parts.append(ps[:, n0:].sum(axis=1))        # rows rt1
    S_dev = np.concatenate(S_parts)

    # Host correction: swap the label column's contribution, O(B) work.
    rows = np.arange(B)
    ct_l_raw = ct[rows, lab].astype(np.float64)
    ct_l = np.clip(ct_l_raw, -1.0 + EPS, 1.0 - EPS)
    m = mg[lab]
    target = ct_l * np.cos(m) - np.sqrt(1.0 - ct_l * ct_l) * np.sin(m)
    z_new = SCALE * target
    S_corr = S_dev - np.exp(SCALE * ct_l_raw - SCALE) + np.exp(z_new - SCALE)
    loss_i = (SCALE + np.log(S_corr)) - z_new
    return np.array(loss_i.mean(), dtype=np.float32)
